# revision 22
# baseline (speedup 1.0000x reference)
"""Trainium2 Bass kernel for dense multi-head causal self-attention.

Problem: hidden_states [2, 2048, 2048], w_qkv [6144, 2048], w_out [2048, 2048],
16 heads x 128 head_dim, causal softmax attention + out projection.

Sharding: tensor-parallel over heads. Each of the 8 cores computes 2 heads:
  - qkv projection for its 768 w_qkv rows (bf16 matmuls; x, w, out all in
    partition-major token-block layouts so every DMA is one contiguous run
    per partition: one 2MB x DMA per token block, prefetched one block
    ahead); q,k produced transposed [d, tok], v produced natural [tok, d]
  - causal attention: scores transposed [k, q], exp on ScalarE (bf16 probs),
    causal mask multiply on GpSimd with the P@V matmul pipelined one k-tile
    behind so mask latency never stalls the PE; probs compressed across
    k-tiles on the DVE (S += pr) so the softmax denominator costs one
    512-row all-ones matmul per block, landing broadcast on every
    partition; normalize = reciprocal_approx_fast + one fused psum*recip
    multiply on the DVE
  - partial out-projection against its 256 w_out columns; bf16 staging
    copies split ScalarE/DVE, quad-batched contiguous output DMAs
Host sums the 8 partial bf16 outputs in f32 (the "all-reduce").
"""

import sys

sys.path.insert(0, "/opt/trn_rl_repo")

import numpy as np

B, T, H, NH, HD = 2, 2048, 2048, 16, 128
TOK = B * T  # 4096
P = 128
NCORES = 8
HPC = NH // NCORES  # heads per core = 2
SCALE = 1.0 / float(np.sqrt(HD))
QB = 512  # query block
KT = H // P  # 16 contraction tiles for qkv
NTB = TOK // QB  # 8 token blocks
NQK = 2 * HPC  # q,k output row-tiles per core
VW = HPC * HD  # v width (both heads) = 256
NM = H // P  # out-projection row tiles = 16

_CACHE = {}


def _build():
    import concourse.bacc as bacc
    import concourse.mybir as mybir
    import concourse.tile as tile

    dt = mybir.dt
    f32 = dt.float32
    bf16 = dt.bfloat16
    AF = mybir.ActivationFunctionType
    ALU = mybir.AluOpType

    nc = bacc.Bacc(None, target_bir_lowering=False, debug=True)
    # token-block-major layouts: one contiguous run per partition per DMA
    xT = nc.dram_tensor("xT", [P, NTB, KT, QB], bf16, kind="ExternalInput")
    wqkvT = nc.dram_tensor("wqkvT", [P, KT, 6 * P], bf16, kind="ExternalInput")
    woutT = nc.dram_tensor("woutT", [P, HPC, H], bf16, kind="ExternalInput")
    tri = nc.dram_tensor("tri", [P, P], bf16, kind="ExternalInput")
    ones = nc.dram_tensor("ones", [P, P], bf16, kind="ExternalInput")
    outT = nc.dram_tensor("outT", [P, NTB, NM, QB], bf16, kind="ExternalOutput")

    with tile.TileContext(nc) as tc:
        with tc.tile_pool(name="const", bufs=1) as constp, \
             tc.tile_pool(name="qk", bufs=1) as qkp:
            tri_sb = constp.tile([P, P], bf16)
            ones_sb = constp.tile([P, P], bf16)

            qT_sb = qkp.tile([P, HPC, TOK], bf16)  # [d, h, tok]
            kT_sb = qkp.tile([P, HPC, TOK], bf16)  # [d, h, tok]
            v_sb = qkp.tile([P, TOK // P, VW], bf16)  # [tok%P, tok//P, h*HD+d]
            wo_sb = qkp.tile([P, HPC, H], bf16)  # long-lived: loads early

            # ---------- Phase 1: qkv projection ----------
            # q,k transposed: psum[o,tok] = w_slice.T @ xT ; v natural:
            # psum[tok,d] = xT_chunk.T @ w_vT
            with tc.tile_pool(name="w1", bufs=1) as w1p, \
                 tc.tile_pool(name="x1", bufs=2) as x1p, \
                 tc.tile_pool(name="ps1", bufs=1, space="PSUM") as ps1, \
                 tc.tile_pool(name="psv", bufs=1, space="PSUM") as psv:
                w_sb = w1p.tile([P, KT, 6 * P], bf16)
                x_tiles = {}

                def load_x(tb):
                    xt = x1p.tile([P, KT, QB], bf16, tag="x", name=f"x_{tb}")
                    nc.sync.dma_start(xt[:], xT[:, tb, :, :])
                    x_tiles[tb] = xt

                # tb0's x and the weights arrive in ko-consumption order on
                # two parallel issue streams (x on the sync sequencer, w and
                # constants on the scalar sequencer) so the PE is never
                # starved by serialized DMA issues
                xt0 = x1p.tile([P, KT, QB], bf16, tag="x", name="x_0")
                x_tiles[0] = xt0
                nc.scalar.dma_start(w_sb[:, 0:1, :], wqkvT[:, 0:1, :])
                nc.sync.dma_start(xt0[:, 0:1, :], xT[:, 0, 0:1, :])
                for g0, g1 in ((1, 3), (3, 5), (5, 8), (8, 11), (11, 14),
                               (14, KT)):
                    nc.sync.dma_start(xt0[:, g0:g1, :], xT[:, 0, g0:g1, :])
                    nc.scalar.dma_start(w_sb[:, g0:g1, :], wqkvT[:, g0:g1, :])
                nc.scalar.dma_start(tri_sb[:], tri[:])
                nc.scalar.dma_start(ones_sb[:], ones[:])
                nc.scalar.dma_start(wo_sb[:], woutT[:])

                for tb in range(NTB):
                    if tb + 1 < NTB:
                        load_x(tb + 1)
                    x_t = x_tiles.pop(tb)
                    ps_qk = [
                        ps1.tile([P, QB], f32, tag=f"psqk{m}", name=f"psqk{m}_{tb}")
                        for m in range(NQK)
                    ]
                    ps_v = [
                        psv.tile([P, VW], f32, tag=f"psv{c}", name=f"psv{c}_{tb}")
                        for c in range(4)
                    ]
                    for ko in range(KT):
                        xk = x_t[:, ko, :]
                        for c in range(4):
                            nc.tensor.matmul(
                                ps_v[c][:],
                                xk[:, c * P:(c + 1) * P],
                                w_sb[:, ko, NQK * P:],
                                start=(ko == 0),
                                stop=(ko == KT - 1),
                            )
                        for m in range(NQK):
                            nc.tensor.matmul(
                                ps_qk[m][:],
                                w_sb[:, ko, m * P:(m + 1) * P],
                                xk[:],
                                start=(ko == 0),
                                stop=(ko == KT - 1),
                            )
                    for c in range(4):
                        nc.scalar.copy(v_sb[:, tb * 4 + c, :], ps_v[c][:])
                    for m in range(NQK):
                        if m < HPC:
                            dst = qT_sb[:, m, tb * QB:(tb + 1) * QB]
                        else:
                            dst = kT_sb[:, m - HPC, tb * QB:(tb + 1) * QB]
                        if tb == NTB - 1:
                            # last block gates the attention phase's PSUM
                            # handoff: split each copy across both engines
                            nc.vector.tensor_copy(dst[:, :QB // 2],
                                                  ps_qk[m][:, :QB // 2])
                            nc.scalar.copy(dst[:, QB // 2:],
                                           ps_qk[m][:, QB // 2:])
                        else:
                            nc.vector.tensor_copy(dst, ps_qk[m][:])

            # ---------- Phases 2+3 (attn_sb reuses phase-1 pool space) ----------
            with tc.tile_pool(name="attn", bufs=1) as attnp:
                attn_sb = attnp.tile([P, HPC, TOK], bf16)  # attn^T [d, h, tok]

                with tc.tile_pool(name="pr", bufs=8) as prp, \
                     tc.tile_pool(name="o3", bufs=6) as o3p, \
                     tc.tile_pool(name="nrm", bufs=2) as nrmp, \
                     tc.tile_pool(name="ps_sc", bufs=3, space="PSUM") as ps_sc, \
                     tc.tile_pool(name="ps_at", bufs=2, space="PSUM") as ps_at, \
                     tc.tile_pool(name="ps_sm", bufs=1, space="PSUM") as ps_sm, \
                     tc.tile_pool(name="ps3", bufs=2, space="PSUM") as ps3:

                    # the tail of each block (last P@V matmul, denominator
                    # matmul, reciprocal, normalize) is deferred into the
                    # next block, after its first scores matmuls are queued,
                    # so the PE never waits on the last probs tile
                    deferred = [None]

                    def flush_tail():
                        if deferred[0] is not None:
                            fin = deferred[0]
                            deferred[0] = None
                            fin()

                    def attn_block(b, h, j):
                        base = b * T
                        q_ap = qT_sb[:, h, base + j * QB: base + (j + 1) * QB]
                        n_k = (j + 1) * (QB // P)
                        attn_ps = ps_at.tile([P, QB], f32, tag="attn",
                                             name=f"at_{b}_{h}_{j}")
                        s_t = nrmp.tile([P, QB], bf16, tag="S",
                                        name=f"S_{b}_{h}_{j}")

                        def koff(kt):
                            diag = kt - j * (QB // P)
                            return diag * P if diag >= 0 else 0

                        def emit_scores(kt):
                            off = koff(kt)
                            sc = ps_sc.tile([P, QB], f32, tag="sc",
                                            name=f"sc_{b}_{h}_{j}_{kt}")
                            nc.tensor.matmul(
                                sc[:, off:],
                                kT_sb[:, h, base + kt * P: base + (kt + 1) * P],
                                q_ap[:, off:],
                                start=True,
                                stop=True,
                            )
                            return sc

                        prs = {}

                        def consume(kt):
                            # P@V matmul + S compression for a masked tile;
                            # runs one k-tile behind the exp/mask producers
                            off = koff(kt)
                            pr = prs.pop(kt)
                            nc.tensor.matmul(
                                attn_ps[:, off:],
                                v_sb[:, b * (T // P) + kt, h * HD:(h + 1) * HD],
                                pr[:, off:],
                                start=(kt == 0),
                                stop=(kt == n_k - 1),
                            )
                            if kt == 0:
                                nc.vector.tensor_copy(s_t[:], pr[:])
                            else:
                                nc.vector.tensor_tensor(
                                    s_t[:, off:], s_t[:, off:], pr[:, off:],
                                    ALU.add,
                                )

                        sc_q = [emit_scores(0)]
                        if n_k > 1:
                            sc_q.append(emit_scores(1))
                        flush_tail()
                        for kt in range(n_k):
                            off = koff(kt)
                            diag = kt - j * (QB // P)
                            pr = prp.tile([P, QB], bf16, tag="pr",
                                          name=f"pr_{b}_{h}_{j}_{kt}")
                            prs[kt] = pr
                            nc.scalar.activation(
                                pr[:, off:], sc_q[kt][:, off:], AF.Exp,
                                scale=SCALE,
                            )
                            if kt + 2 < n_k:
                                sc_q.append(emit_scores(kt + 2))
                            if diag >= 0:
                                nc.gpsimd.tensor_tensor(
                                    pr[:, off:off + P], pr[:, off:off + P],
                                    tri_sb[:], ALU.mult,
                                )
                            if kt >= 1:
                                consume(kt - 1)

                        def fin():
                            consume(n_k - 1)
                            sums_ps = ps_sm.tile([P, QB], f32, tag="sums",
                                                 name=f"sm_{b}_{h}_{j}")
                            nc.tensor.matmul(
                                sums_ps[:], ones_sb[:], s_t[:],
                                start=True, stop=True,
                            )
                            rcp = nrmp.tile([P, QB], f32, tag="rcp",
                                            name=f"rcp_{b}_{h}_{j}")
                            nc.vector.reciprocal_approx_fast(rcp[:], sums_ps[:])
                            nc.vector.tensor_tensor(
                                attn_sb[:, h,
                                        base + j * QB: base + (j + 1) * QB],
                                attn_ps[:], rcp[:], ALU.mult,
                            )

                        deferred[0] = fin

                    def outproj_quad(b, j, mg, final=False):
                        tb = b * (T // QB) + j
                        o_q = o3p.tile([P, 4, QB], bf16, tag="osb",
                                       name=f"osb_{tb}_{mg}")
                        for mi in range(4):
                            m = mg * 4 + mi
                            if final and m % 2 == 1:
                                # attention is done: rotate through the freed
                                # scores banks too so matmuls never wait on
                                # the staging copies
                                ps = ps_sc.tile([P, QB], f32, tag="sc",
                                                name=f"out_{tb}_{m}")
                            else:
                                ps = ps3.tile([P, QB], f32, tag="out",
                                              name=f"out_{tb}_{m}")
                            for ko in range(HPC):
                                nc.tensor.matmul(
                                    ps[:],
                                    wo_sb[:, ko, m * P:(m + 1) * P],
                                    attn_sb[:, ko, tb * QB:(tb + 1) * QB],
                                    start=(ko == 0),
                                    stop=(ko == HPC - 1),
                                )
                            if m % 2 == 0:
                                nc.scalar.copy(o_q[:, mi, :], ps[:])
                            else:
                                nc.vector.tensor_copy(o_q[:, mi, :], ps[:])
                        if final:
                            # smaller writes at the very end so the last
                            # transfer starts (and drains) earlier
                            for hq in range(2):
                                nc.sync.dma_start(
                                    outT[:, tb,
                                         mg * 4 + hq * 2: mg * 4 + hq * 2 + 2,
                                         :],
                                    o_q[:, hq * 2:(hq + 1) * 2, :],
                                )
                        else:
                            nc.sync.dma_start(
                                outT[:, tb, mg * 4:(mg + 1) * 4, :],
                                o_q[:],
                            )

                    # fused per (b, j): both heads' attention + normalize; the
                    # out-projection of the PREVIOUS block is emitted after the
                    # next attention so its matmuls hide the normalize latency
                    # each block's quads are spread over the next two
                    # half-block boundaries (two quads after every head's
                    # attention) so the exp pipeline is never starved by a
                    # long out-projection stretch, while the normalize chain
                    # of a block always has a full head-block of slack
                    # before its quads hit the in-order PE queue
                    hist = []
                    for b in range(B):
                        for j in range(T // QB):
                            attn_block(b, 0, j)
                            if len(hist) >= 2:
                                outproj_quad(*hist[-2], 2)
                                outproj_quad(*hist[-2], 3)
                            attn_block(b, 1, j)
                            if len(hist) >= 1:
                                outproj_quad(*hist[-1], 0)
                                outproj_quad(*hist[-1], 1)
                            hist.append((b, j))
                    flush_tail()
                    outproj_quad(*hist[-2], 2)
                    outproj_quad(*hist[-2], 3)
                    for mg in range(NM // 4):
                        outproj_quad(*hist[-1], mg, final=True)
    nc.finalize()
    return nc


def _host_inputs(hidden_states, w_qkv, w_out):
    import ml_dtypes

    BF16 = np.dtype(ml_dtypes.bfloat16)
    x = np.asarray(hidden_states, dtype=np.float32).reshape(TOK, H)
    w_qkv = np.asarray(w_qkv, dtype=np.float32)
    w_out = np.asarray(w_out, dtype=np.float32)

    # [P, NTB, KT, QB]: xT4[p, tb, ko, q] = x.T[ko*P + p, tb*QB + q]
    xT4 = np.ascontiguousarray(
        x.T.reshape(KT, P, NTB, QB).transpose(1, 2, 0, 3)
    ).astype(BF16)

    # lower-triangle-inclusive mask for the diagonal 128x128 strip
    tri = (np.arange(P)[:, None] <= np.arange(P)[None, :]).astype(BF16)
    ones = np.ones((P, P), dtype=BF16)

    in_maps = []
    for c in range(NCORES):
        heads = [HPC * c + i for i in range(HPC)]
        rows = []
        for sec in range(2):  # q, k sections
            for hh in heads:
                rows.append(w_qkv[sec * H + hh * HD: sec * H + (hh + 1) * HD])
        for hh in heads:  # v section
            rows.append(w_qkv[2 * H + hh * HD: 2 * H + (hh + 1) * HD])
        w_slice = np.concatenate(rows, axis=0)  # [768, H]
        wqkvT3 = np.ascontiguousarray(
            w_slice.T.reshape(KT, P, 6 * P).transpose(1, 0, 2)
        ).astype(BF16)  # [P, KT, 768]
        cols = np.concatenate([np.arange(hh * HD, (hh + 1) * HD) for hh in heads])
        woutT3 = np.ascontiguousarray(
            w_out[:, cols].T.reshape(HPC, P, H).transpose(1, 0, 2)
        ).astype(BF16)  # [P, HPC, H]
        in_maps.append({
            "xT": xT4,
            "wqkvT": wqkvT3,
            "woutT": woutT3,
            "tri": tri,
            "ones": ones,
        })
    return in_maps


def _assemble(res):
    # outT is [P, NTB, NM, QB]; row m*P+p, col tb*QB+q of the logical
    # [H, TOK] partial lives at outT[p, tb, m, q]
    acc = res.results[0]["outT"].astype(np.float32)
    for c in range(1, NCORES):
        acc = acc + res.results[c]["outT"].astype(np.float32)
    full = acc.transpose(2, 0, 1, 3).reshape(H, TOK)  # [H, TOK]
    return np.ascontiguousarray(full.T).reshape(B, T, H)


def _run(in_maps, trace=False):
    from concourse.bass_utils import run_bass_kernel_spmd

    if "nc" not in _CACHE:
        _CACHE["nc"] = _build()
    return run_bass_kernel_spmd(
        _CACHE["nc"], in_maps, core_ids=list(range(NCORES)), trace=trace
    )


def kernel(hidden_states, w_qkv, w_out):
    in_maps = _host_inputs(hidden_states, w_qkv, w_out)
    res = _run(in_maps)
    return _assemble(res).astype(np.float32)


# revision 23
# speedup vs baseline: 1.1973x; 1.1973x over previous
"""Trainium2 Bass kernel for dense multi-head causal self-attention.

Problem: hidden_states [2, 2048, 2048], w_qkv [6144, 2048], w_out [2048, 2048],
16 heads x 128 head_dim, causal softmax attention + out projection.

Sharding: tensor-parallel over heads. Each of the 8 cores computes 2 heads:
  - qkv projection for its 768 w_qkv rows (bf16 matmuls; x, w, out all in
    partition-major token-block layouts so every DMA is one contiguous run
    per partition: one 2MB x DMA per token block, prefetched one block
    ahead); q,k produced transposed [d, tok], v produced natural [tok, d]
  - causal attention: scores transposed [k, q], exp on ScalarE (bf16 probs),
    causal mask multiply on GpSimd with the P@V matmul pipelined one k-tile
    behind so mask latency never stalls the PE; probs compressed across
    k-tiles on the DVE (S += pr) so the softmax denominator costs one
    512-row all-ones matmul per block, landing broadcast on every
    partition; normalize = reciprocal_approx_fast + one fused psum*recip
    multiply on the DVE
  - partial out-projection against its 256 w_out columns; bf16 staging
    copies split ScalarE/DVE, quad-batched contiguous output DMAs
Host sums the 8 partial bf16 outputs in f32 (the "all-reduce").
"""

import sys

sys.path.insert(0, "/opt/trn_rl_repo")

import numpy as np

B, T, H, NH, HD = 2, 2048, 2048, 16, 128
TOK = B * T  # 4096
P = 128
NCORES = 8
HPC = NH // NCORES  # heads per core = 2
SCALE = 1.0 / float(np.sqrt(HD))
QB = 512  # query block
KT = H // P  # 16 contraction tiles for qkv
NTB = TOK // QB  # 8 token blocks
NQK = 2 * HPC  # q,k output row-tiles per core
VW = HPC * HD  # v width (both heads) = 256
NM = H // P  # out-projection row tiles = 16

_CACHE = {}


def _build():
    import concourse.bacc as bacc
    import concourse.mybir as mybir
    import concourse.tile as tile

    dt = mybir.dt
    f32 = dt.float32
    bf16 = dt.bfloat16
    AF = mybir.ActivationFunctionType
    ALU = mybir.AluOpType

    nc = bacc.Bacc(None, target_bir_lowering=False, debug=True)
    # token-block-major layouts: one contiguous run per partition per DMA
    xT = nc.dram_tensor("xT", [P, NTB, KT, QB], bf16, kind="ExternalInput")
    wqkvT = nc.dram_tensor("wqkvT", [P, KT, 6 * P], bf16, kind="ExternalInput")
    woutT = nc.dram_tensor("woutT", [P, HPC, H], bf16, kind="ExternalInput")
    tri = nc.dram_tensor("tri", [P, P], bf16, kind="ExternalInput")
    ones = nc.dram_tensor("ones", [P, P], bf16, kind="ExternalInput")
    outT = nc.dram_tensor("outT", [P, NTB, NM, QB], bf16, kind="ExternalOutput")

    with tile.TileContext(nc) as tc:
        with tc.tile_pool(name="const", bufs=1) as constp, \
             tc.tile_pool(name="qk", bufs=1) as qkp:
            tri_sb = constp.tile([P, P], bf16)
            ones_sb = constp.tile([P, P], bf16)

            qT_sb = qkp.tile([P, HPC, TOK], bf16)  # [d, h, tok]
            kT_sb = qkp.tile([P, HPC, TOK], bf16)  # [d, h, tok]
            v_sb = qkp.tile([P, TOK // P, VW], bf16)  # [tok%P, tok//P, h*HD+d]
            wo_sb = qkp.tile([P, HPC, H], bf16)  # long-lived: loads early

            # ---------- Phase 1: qkv projection ----------
            # q,k transposed: psum[o,tok] = w_slice.T @ xT ; v natural:
            # psum[tok,d] = xT_chunk.T @ w_vT
            with tc.tile_pool(name="w1", bufs=1) as w1p, \
                 tc.tile_pool(name="x1", bufs=2) as x1p, \
                 tc.tile_pool(name="ps1", bufs=1, space="PSUM") as ps1, \
                 tc.tile_pool(name="psv", bufs=1, space="PSUM") as psv:
                w_sb = w1p.tile([P, KT, 6 * P], bf16)
                x_tiles = {}

                def load_x(tb):
                    xt = x1p.tile([P, KT, QB], bf16, tag="x", name=f"x_{tb}")
                    nc.sync.dma_start(xt[:], xT[:, tb, :, :])
                    x_tiles[tb] = xt

                # tb0's x and the weights arrive in ko-consumption order on
                # two parallel issue streams (x on the sync sequencer, w and
                # constants on the scalar sequencer) so the PE is never
                # starved by serialized DMA issues
                xt0 = x1p.tile([P, KT, QB], bf16, tag="x", name="x_0")
                x_tiles[0] = xt0
                nc.scalar.dma_start(w_sb[:, 0:1, :], wqkvT[:, 0:1, :])
                nc.sync.dma_start(xt0[:, 0:1, :], xT[:, 0, 0:1, :])
                for g0, g1 in ((1, 3), (3, 5), (5, 8), (8, 11), (11, 14),
                               (14, KT)):
                    nc.sync.dma_start(xt0[:, g0:g1, :], xT[:, 0, g0:g1, :])
                    nc.scalar.dma_start(w_sb[:, g0:g1, :], wqkvT[:, g0:g1, :])
                nc.scalar.dma_start(tri_sb[:], tri[:])
                nc.scalar.dma_start(ones_sb[:], ones[:])
                nc.scalar.dma_start(wo_sb[:], woutT[:])

                for tb in range(NTB):
                    if tb + 1 < NTB:
                        load_x(tb + 1)
                    x_t = x_tiles.pop(tb)
                    ps_qk = [
                        ps1.tile([P, QB], f32, tag=f"psqk{m}", name=f"psqk{m}_{tb}")
                        for m in range(NQK)
                    ]
                    ps_v = [
                        psv.tile([P, VW], f32, tag=f"psv{c}", name=f"psv{c}_{tb}")
                        for c in range(4)
                    ]
                    for ko in range(KT):
                        xk = x_t[:, ko, :]
                        for c in range(4):
                            nc.tensor.matmul(
                                ps_v[c][:],
                                xk[:, c * P:(c + 1) * P],
                                w_sb[:, ko, NQK * P:],
                                start=(ko == 0),
                                stop=(ko == KT - 1),
                            )
                        for m in range(NQK):
                            nc.tensor.matmul(
                                ps_qk[m][:],
                                w_sb[:, ko, m * P:(m + 1) * P],
                                xk[:],
                                start=(ko == 0),
                                stop=(ko == KT - 1),
                            )
                    for c in range(4):
                        nc.scalar.copy(v_sb[:, tb * 4 + c, :], ps_v[c][:])
                    for m in range(NQK):
                        if m < HPC:
                            dst = qT_sb[:, m, tb * QB:(tb + 1) * QB]
                        else:
                            dst = kT_sb[:, m - HPC, tb * QB:(tb + 1) * QB]
                        if tb == NTB - 1:
                            # last block gates the attention phase's PSUM
                            # handoff: split each copy across both engines
                            nc.vector.tensor_copy(dst[:, :QB // 2],
                                                  ps_qk[m][:, :QB // 2])
                            nc.scalar.copy(dst[:, QB // 2:],
                                           ps_qk[m][:, QB // 2:])
                        else:
                            nc.vector.tensor_copy(dst, ps_qk[m][:])

            # ---------- Phases 2+3 (attn_sb reuses phase-1 pool space) ----------
            with tc.tile_pool(name="attn", bufs=1) as attnp:
                attn_sb = attnp.tile([P, HPC, TOK], bf16)  # attn^T [d, h, tok]

                with tc.tile_pool(name="pr", bufs=8) as prp, \
                     tc.tile_pool(name="o3", bufs=6) as o3p, \
                     tc.tile_pool(name="nrm", bufs=2) as nrmp, \
                     tc.tile_pool(name="ps_sc", bufs=3, space="PSUM") as ps_sc, \
                     tc.tile_pool(name="ps_at", bufs=2, space="PSUM") as ps_at, \
                     tc.tile_pool(name="ps_sm", bufs=1, space="PSUM") as ps_sm, \
                     tc.tile_pool(name="ps3", bufs=2, space="PSUM") as ps3:

                    # the tail of each block (last P@V matmul, denominator
                    # matmul, reciprocal, normalize) is deferred into the
                    # next block, after its first scores matmuls are queued,
                    # so the PE never waits on the last probs tile
                    deferred = [None]

                    def flush_tail():
                        if deferred[0] is not None:
                            fin = deferred[0]
                            deferred[0] = None
                            fin()

                    def attn_block(b, h, j):
                        base = b * T
                        q_ap = qT_sb[:, h, base + j * QB: base + (j + 1) * QB]
                        n_k = (j + 1) * (QB // P)
                        attn_ps = ps_at.tile([P, QB], f32, tag="attn",
                                             name=f"at_{b}_{h}_{j}")
                        s_t = nrmp.tile([P, QB], bf16, tag="S",
                                        name=f"S_{b}_{h}_{j}")

                        def koff(kt):
                            diag = kt - j * (QB // P)
                            return diag * P if diag >= 0 else 0

                        def emit_scores(kt):
                            off = koff(kt)
                            sc = ps_sc.tile([P, QB], f32, tag="sc",
                                            name=f"sc_{b}_{h}_{j}_{kt}")
                            nc.tensor.matmul(
                                sc[:, off:],
                                kT_sb[:, h, base + kt * P: base + (kt + 1) * P],
                                q_ap[:, off:],
                                start=True,
                                stop=True,
                            )
                            return sc

                        prs = {}

                        def consume(kt):
                            # P@V matmul + S compression for a masked tile;
                            # runs one k-tile behind the exp/mask producers
                            off = koff(kt)
                            pr = prs.pop(kt)
                            nc.tensor.matmul(
                                attn_ps[:, off:],
                                v_sb[:, b * (T // P) + kt, h * HD:(h + 1) * HD],
                                pr[:, off:],
                                start=(kt == 0),
                                stop=(kt == n_k - 1),
                            )
                            if kt == 0:
                                nc.vector.tensor_copy(s_t[:], pr[:])
                            else:
                                nc.vector.tensor_tensor(
                                    s_t[:, off:], s_t[:, off:], pr[:, off:],
                                    ALU.add,
                                )

                        sc_q = [emit_scores(0)]
                        if n_k > 1:
                            sc_q.append(emit_scores(1))
                        flush_tail()
                        for kt in range(n_k):
                            off = koff(kt)
                            diag = kt - j * (QB // P)
                            pr = prp.tile([P, QB], bf16, tag="pr",
                                          name=f"pr_{b}_{h}_{j}_{kt}")
                            prs[kt] = pr
                            nc.scalar.activation(
                                pr[:, off:], sc_q[kt][:, off:], AF.Exp,
                                scale=SCALE,
                            )
                            if kt + 2 < n_k:
                                sc_q.append(emit_scores(kt + 2))
                            if diag >= 0:
                                nc.gpsimd.tensor_tensor(
                                    pr[:, off:off + P], pr[:, off:off + P],
                                    tri_sb[:], ALU.mult,
                                )
                            if kt >= 1:
                                consume(kt - 1)

                        def fin():
                            consume(n_k - 1)
                            sums_ps = ps_sm.tile([P, QB], f32, tag="sums",
                                                 name=f"sm_{b}_{h}_{j}")
                            nc.tensor.matmul(
                                sums_ps[:], ones_sb[:], s_t[:],
                                start=True, stop=True,
                            )
                            rcp = nrmp.tile([P, QB], f32, tag="rcp",
                                            name=f"rcp_{b}_{h}_{j}")
                            nc.vector.reciprocal_approx_fast(rcp[:], sums_ps[:])
                            nc.vector.tensor_tensor(
                                attn_sb[:, h,
                                        base + j * QB: base + (j + 1) * QB],
                                attn_ps[:], rcp[:], ALU.mult,
                            )

                        deferred[0] = fin

                    def outproj_quad(b, j, mg, final=False):
                        tb = b * (T // QB) + j
                        o_q = o3p.tile([P, 4, QB], bf16, tag="osb",
                                       name=f"osb_{tb}_{mg}")
                        for mi in range(4):
                            m = mg * 4 + mi
                            if final and m % 2 == 1:
                                # attention is done: rotate through the freed
                                # scores banks too so matmuls never wait on
                                # the staging copies
                                ps = ps_sc.tile([P, QB], f32, tag="sc",
                                                name=f"out_{tb}_{m}")
                            else:
                                ps = ps3.tile([P, QB], f32, tag="out",
                                              name=f"out_{tb}_{m}")
                            for ko in range(HPC):
                                nc.tensor.matmul(
                                    ps[:],
                                    wo_sb[:, ko, m * P:(m + 1) * P],
                                    attn_sb[:, ko, tb * QB:(tb + 1) * QB],
                                    start=(ko == 0),
                                    stop=(ko == HPC - 1),
                                )
                            if m % 2 == 0:
                                nc.scalar.copy(o_q[:, mi, :], ps[:])
                            else:
                                nc.vector.tensor_copy(o_q[:, mi, :], ps[:])
                        if final:
                            # smaller writes at the very end so the last
                            # transfer starts (and drains) earlier
                            for hq in range(2):
                                nc.sync.dma_start(
                                    outT[:, tb,
                                         mg * 4 + hq * 2: mg * 4 + hq * 2 + 2,
                                         :],
                                    o_q[:, hq * 2:(hq + 1) * 2, :],
                                )
                        else:
                            nc.sync.dma_start(
                                outT[:, tb, mg * 4:(mg + 1) * 4, :],
                                o_q[:],
                            )

                    # fused per (b, j): both heads' attention + normalize; the
                    # out-projection of the PREVIOUS block is emitted after the
                    # next attention so its matmuls hide the normalize latency
                    pending = None
                    for b in range(B):
                        for j in range(T // QB):
                            for h in range(HPC):
                                attn_block(b, h, j)
                            if pending is not None:
                                for mg in range(NM // 4):
                                    outproj_quad(*pending, mg)
                            pending = (b, j)
                    flush_tail()
                    for mg in range(NM // 4):
                        outproj_quad(*pending, mg, final=True)
    nc.finalize()
    return nc


def _host_inputs(hidden_states, w_qkv, w_out):
    import ml_dtypes

    BF16 = np.dtype(ml_dtypes.bfloat16)
    x = np.asarray(hidden_states, dtype=np.float32).reshape(TOK, H)
    w_qkv = np.asarray(w_qkv, dtype=np.float32)
    w_out = np.asarray(w_out, dtype=np.float32)

    # [P, NTB, KT, QB]: xT4[p, tb, ko, q] = x.T[ko*P + p, tb*QB + q]
    xT4 = np.ascontiguousarray(
        x.T.reshape(KT, P, NTB, QB).transpose(1, 2, 0, 3)
    ).astype(BF16)

    # lower-triangle-inclusive mask for the diagonal 128x128 strip
    tri = (np.arange(P)[:, None] <= np.arange(P)[None, :]).astype(BF16)
    ones = np.ones((P, P), dtype=BF16)

    in_maps = []
    for c in range(NCORES):
        heads = [HPC * c + i for i in range(HPC)]
        rows = []
        for sec in range(2):  # q, k sections
            for hh in heads:
                rows.append(w_qkv[sec * H + hh * HD: sec * H + (hh + 1) * HD])
        for hh in heads:  # v section
            rows.append(w_qkv[2 * H + hh * HD: 2 * H + (hh + 1) * HD])
        w_slice = np.concatenate(rows, axis=0)  # [768, H]
        wqkvT3 = np.ascontiguousarray(
            w_slice.T.reshape(KT, P, 6 * P).transpose(1, 0, 2)
        ).astype(BF16)  # [P, KT, 768]
        cols = np.concatenate([np.arange(hh * HD, (hh + 1) * HD) for hh in heads])
        woutT3 = np.ascontiguousarray(
            w_out[:, cols].T.reshape(HPC, P, H).transpose(1, 0, 2)
        ).astype(BF16)  # [P, HPC, H]
        in_maps.append({
            "xT": xT4,
            "wqkvT": wqkvT3,
            "woutT": woutT3,
            "tri": tri,
            "ones": ones,
        })
    return in_maps


def _assemble(res):
    # outT is [P, NTB, NM, QB]; row m*P+p, col tb*QB+q of the logical
    # [H, TOK] partial lives at outT[p, tb, m, q]
    acc = res.results[0]["outT"].astype(np.float32)
    for c in range(1, NCORES):
        acc = acc + res.results[c]["outT"].astype(np.float32)
    full = acc.transpose(2, 0, 1, 3).reshape(H, TOK)  # [H, TOK]
    return np.ascontiguousarray(full.T).reshape(B, T, H)


def _run(in_maps, trace=False):
    from concourse.bass_utils import run_bass_kernel_spmd

    if "nc" not in _CACHE:
        _CACHE["nc"] = _build()
    return run_bass_kernel_spmd(
        _CACHE["nc"], in_maps, core_ids=list(range(NCORES)), trace=trace
    )


def kernel(hidden_states, w_qkv, w_out):
    in_maps = _host_inputs(hidden_states, w_qkv, w_out)
    res = _run(in_maps)
    return _assemble(res).astype(np.float32)


# revision 25
# speedup vs baseline: 1.1987x; 1.0012x over previous
"""Trainium2 Bass kernel for dense multi-head causal self-attention.

Problem: hidden_states [2, 2048, 2048], w_qkv [6144, 2048], w_out [2048, 2048],
16 heads x 128 head_dim, causal softmax attention + out projection.

Sharding: tensor-parallel over heads. Each of the 8 cores computes 2 heads:
  - qkv projection for its 768 w_qkv rows (bf16 matmuls; x, w, out all in
    partition-major token-block layouts so every DMA is one contiguous run
    per partition: one 2MB x DMA per token block, prefetched one block
    ahead); q,k produced transposed [d, tok], v produced natural [tok, d]
  - causal attention: scores transposed [k, q], exp on ScalarE (bf16 probs),
    causal mask multiply on GpSimd with the P@V matmul pipelined one k-tile
    behind so mask latency never stalls the PE; probs compressed across
    k-tiles on the DVE (S += pr) so the softmax denominator costs one
    512-row all-ones matmul per block, landing broadcast on every
    partition; normalize = reciprocal_approx_fast + one fused psum*recip
    multiply on the DVE
  - partial out-projection against its 256 w_out columns; bf16 staging
    copies split ScalarE/DVE, quad-batched contiguous output DMAs
Host sums the 8 partial bf16 outputs in f32 (the "all-reduce").
"""

import sys

sys.path.insert(0, "/opt/trn_rl_repo")

import numpy as np

B, T, H, NH, HD = 2, 2048, 2048, 16, 128
TOK = B * T  # 4096
P = 128
NCORES = 8
HPC = NH // NCORES  # heads per core = 2
SCALE = 1.0 / float(np.sqrt(HD))
QB = 512  # query block
KT = H // P  # 16 contraction tiles for qkv
NTB = TOK // QB  # 8 token blocks
NQK = 2 * HPC  # q,k output row-tiles per core
VW = HPC * HD  # v width (both heads) = 256
NM = H // P  # out-projection row tiles = 16

_CACHE = {}


def _build():
    import concourse.bacc as bacc
    import concourse.mybir as mybir
    import concourse.tile as tile

    dt = mybir.dt
    f32 = dt.float32
    bf16 = dt.bfloat16
    AF = mybir.ActivationFunctionType
    ALU = mybir.AluOpType

    nc = bacc.Bacc(None, target_bir_lowering=False, debug=True)
    # token-block-major layouts: one contiguous run per partition per DMA
    xT = nc.dram_tensor("xT", [P, NTB, KT, QB], bf16, kind="ExternalInput")
    wqkvT = nc.dram_tensor("wqkvT", [P, KT, 6 * P], bf16, kind="ExternalInput")
    woutT = nc.dram_tensor("woutT", [P, HPC, H], bf16, kind="ExternalInput")
    tri = nc.dram_tensor("tri", [P, P], bf16, kind="ExternalInput")
    ones = nc.dram_tensor("ones", [P, P], bf16, kind="ExternalInput")
    outT = nc.dram_tensor("outT", [P, NTB, NM, QB], bf16, kind="ExternalOutput")

    with tile.TileContext(nc) as tc:
        with tc.tile_pool(name="const", bufs=1) as constp, \
             tc.tile_pool(name="qk", bufs=1) as qkp:
            tri_sb = constp.tile([P, P], bf16)
            ones_sb = constp.tile([P, P], bf16)

            qT_sb = qkp.tile([P, HPC, TOK], bf16)  # [d, h, tok]
            kT_sb = qkp.tile([P, HPC, TOK], bf16)  # [d, h, tok]
            v_sb = qkp.tile([P, TOK // P, VW], bf16)  # [tok%P, tok//P, h*HD+d]
            wo_sb = qkp.tile([P, HPC, H], bf16)  # long-lived: loads early

            # ---------- Phase 1: qkv projection ----------
            # q,k transposed: psum[o,tok] = w_slice.T @ xT ; v natural:
            # psum[tok,d] = xT_chunk.T @ w_vT
            with tc.tile_pool(name="w1", bufs=1) as w1p, \
                 tc.tile_pool(name="x1", bufs=2) as x1p, \
                 tc.tile_pool(name="ps1", bufs=1, space="PSUM") as ps1, \
                 tc.tile_pool(name="psv", bufs=1, space="PSUM") as psv:
                w_sb = w1p.tile([P, KT, 6 * P], bf16)
                x_tiles = {}

                def load_x(tb):
                    xt = x1p.tile([P, KT, QB], bf16, tag="x", name=f"x_{tb}")
                    nc.sync.dma_start(xt[:], xT[:, tb, :, :])
                    x_tiles[tb] = xt

                # tb0's x and the weights arrive in ko-consumption order on
                # two parallel issue streams (x on the sync sequencer, w and
                # constants on the scalar sequencer) so the PE is never
                # starved by serialized DMA issues
                xt0 = x1p.tile([P, KT, QB], bf16, tag="x", name="x_0")
                x_tiles[0] = xt0
                nc.scalar.dma_start(w_sb[:, 0:1, :], wqkvT[:, 0:1, :])
                nc.sync.dma_start(xt0[:, 0:1, :], xT[:, 0, 0:1, :])
                for g0, g1 in ((1, 3), (3, 5), (5, 8), (8, 11), (11, 14),
                               (14, KT)):
                    nc.sync.dma_start(xt0[:, g0:g1, :], xT[:, 0, g0:g1, :])
                    nc.scalar.dma_start(w_sb[:, g0:g1, :], wqkvT[:, g0:g1, :])
                nc.scalar.dma_start(tri_sb[:], tri[:])
                nc.scalar.dma_start(ones_sb[:], ones[:])
                nc.scalar.dma_start(wo_sb[:], woutT[:])

                for tb in range(NTB):
                    if tb + 1 < NTB:
                        load_x(tb + 1)
                    x_t = x_tiles.pop(tb)
                    ps_qk = [
                        ps1.tile([P, QB], f32, tag=f"psqk{m}", name=f"psqk{m}_{tb}")
                        for m in range(NQK)
                    ]
                    ps_v = [
                        psv.tile([P, VW], f32, tag=f"psv{c}", name=f"psv{c}_{tb}")
                        for c in range(4)
                    ]
                    for ko in range(KT):
                        xk = x_t[:, ko, :]
                        for c in range(4):
                            nc.tensor.matmul(
                                ps_v[c][:],
                                xk[:, c * P:(c + 1) * P],
                                w_sb[:, ko, NQK * P:],
                                start=(ko == 0),
                                stop=(ko == KT - 1),
                            )
                        for m in range(NQK):
                            nc.tensor.matmul(
                                ps_qk[m][:],
                                w_sb[:, ko, m * P:(m + 1) * P],
                                xk[:],
                                start=(ko == 0),
                                stop=(ko == KT - 1),
                            )
                    for c in range(4):
                        nc.scalar.copy(v_sb[:, tb * 4 + c, :], ps_v[c][:])
                    for m in range(NQK):
                        if m < HPC:
                            dst = qT_sb[:, m, tb * QB:(tb + 1) * QB]
                        else:
                            dst = kT_sb[:, m - HPC, tb * QB:(tb + 1) * QB]
                        if tb == NTB - 1:
                            # last block gates the attention phase's PSUM
                            # handoff: split each copy across both engines
                            nc.vector.tensor_copy(dst[:, :QB // 2],
                                                  ps_qk[m][:, :QB // 2])
                            nc.scalar.copy(dst[:, QB // 2:],
                                           ps_qk[m][:, QB // 2:])
                        else:
                            nc.vector.tensor_copy(dst, ps_qk[m][:])

            # ---------- Phases 2+3 (attn_sb reuses phase-1 pool space) ----------
            with tc.tile_pool(name="attn", bufs=1) as attnp:
                attn_sb = attnp.tile([P, HPC, TOK], bf16)  # attn^T [d, h, tok]

                with tc.tile_pool(name="pr", bufs=8) as prp, \
                     tc.tile_pool(name="o3", bufs=6) as o3p, \
                     tc.tile_pool(name="nrm", bufs=2) as nrmp, \
                     tc.tile_pool(name="ps_sc", bufs=3, space="PSUM") as ps_sc, \
                     tc.tile_pool(name="ps_at", bufs=2, space="PSUM") as ps_at, \
                     tc.tile_pool(name="ps_sm", bufs=1, space="PSUM") as ps_sm, \
                     tc.tile_pool(name="ps3", bufs=2, space="PSUM") as ps3:

                    # the tail of each block (last P@V matmul, denominator
                    # matmul, reciprocal, normalize) is deferred into the
                    # next block, after its first scores matmuls are queued,
                    # so the PE never waits on the last probs tile
                    deferred = [None]

                    def flush_tail():
                        if deferred[0] is not None:
                            fin = deferred[0]
                            deferred[0] = None
                            fin()

                    def attn_block(b, h, j):
                        base = b * T
                        q_ap = qT_sb[:, h, base + j * QB: base + (j + 1) * QB]
                        n_k = (j + 1) * (QB // P)
                        attn_ps = ps_at.tile([P, QB], f32, tag="attn",
                                             name=f"at_{b}_{h}_{j}")
                        s_t = nrmp.tile([P, QB], bf16, tag="S",
                                        name=f"S_{b}_{h}_{j}")

                        def koff(kt):
                            diag = kt - j * (QB // P)
                            return diag * P if diag >= 0 else 0

                        def emit_scores(kt):
                            off = koff(kt)
                            sc = ps_sc.tile([P, QB], f32, tag="sc",
                                            name=f"sc_{b}_{h}_{j}_{kt}")
                            nc.tensor.matmul(
                                sc[:, off:],
                                kT_sb[:, h, base + kt * P: base + (kt + 1) * P],
                                q_ap[:, off:],
                                start=True,
                                stop=True,
                            )
                            return sc

                        prs = {}

                        def consume(kt):
                            # P@V matmul + S compression for a masked tile;
                            # runs one k-tile behind the exp/mask producers.
                            # The final tile skips the DVE add: it joins the
                            # denominator as a second sums-matmul accumulation
                            # call, so the PE never waits on the add backlog.
                            off = koff(kt)
                            pr = prs[kt] if kt == n_k - 1 else prs.pop(kt)
                            nc.tensor.matmul(
                                attn_ps[:, off:],
                                v_sb[:, b * (T // P) + kt, h * HD:(h + 1) * HD],
                                pr[:, off:],
                                start=(kt == 0),
                                stop=(kt == n_k - 1),
                            )
                            if kt == 0:
                                nc.vector.tensor_copy(s_t[:], pr[:])
                            elif kt < n_k - 1:
                                nc.vector.tensor_tensor(
                                    s_t[:, off:], s_t[:, off:], pr[:, off:],
                                    ALU.add,
                                )

                        sc_q = [emit_scores(0)]
                        if n_k > 1:
                            sc_q.append(emit_scores(1))
                        flush_tail()
                        for kt in range(n_k):
                            off = koff(kt)
                            diag = kt - j * (QB // P)
                            pr = prp.tile([P, QB], bf16, tag="pr",
                                          name=f"pr_{b}_{h}_{j}_{kt}")
                            prs[kt] = pr
                            nc.scalar.activation(
                                pr[:, off:], sc_q[kt][:, off:], AF.Exp,
                                scale=SCALE,
                            )
                            if kt + 2 < n_k:
                                sc_q.append(emit_scores(kt + 2))
                            if diag >= 0:
                                nc.gpsimd.tensor_tensor(
                                    pr[:, off:off + P], pr[:, off:off + P],
                                    tri_sb[:], ALU.mult,
                                )
                            if kt >= 1:
                                consume(kt - 1)

                        def fin():
                            consume(n_k - 1)
                            last_off = koff(n_k - 1)
                            sums_ps = ps_sm.tile([P, QB], f32, tag="sums",
                                                 name=f"sm_{b}_{h}_{j}")
                            nc.tensor.matmul(
                                sums_ps[:], ones_sb[:], s_t[:],
                                start=True, stop=False,
                            )
                            nc.tensor.matmul(
                                sums_ps[:, last_off:], ones_sb[:],
                                prs.pop(n_k - 1)[:, last_off:],
                                start=False, stop=True,
                            )
                            rcp = nrmp.tile([P, QB], f32, tag="rcp",
                                            name=f"rcp_{b}_{h}_{j}")
                            nc.vector.reciprocal_approx_fast(rcp[:], sums_ps[:])
                            nc.vector.tensor_tensor(
                                attn_sb[:, h,
                                        base + j * QB: base + (j + 1) * QB],
                                attn_ps[:], rcp[:], ALU.mult,
                            )

                        deferred[0] = fin

                    def outproj_quad(b, j, mg, final=False):
                        tb = b * (T // QB) + j
                        o_q = o3p.tile([P, 4, QB], bf16, tag="osb",
                                       name=f"osb_{tb}_{mg}")
                        for mi in range(4):
                            m = mg * 4 + mi
                            if final and m % 2 == 1:
                                # attention is done: rotate through the freed
                                # scores banks too so matmuls never wait on
                                # the staging copies
                                ps = ps_sc.tile([P, QB], f32, tag="sc",
                                                name=f"out_{tb}_{m}")
                            else:
                                ps = ps3.tile([P, QB], f32, tag="out",
                                              name=f"out_{tb}_{m}")
                            for ko in range(HPC):
                                nc.tensor.matmul(
                                    ps[:],
                                    wo_sb[:, ko, m * P:(m + 1) * P],
                                    attn_sb[:, ko, tb * QB:(tb + 1) * QB],
                                    start=(ko == 0),
                                    stop=(ko == HPC - 1),
                                )
                            if m % 2 == 0:
                                nc.scalar.copy(o_q[:, mi, :], ps[:])
                            else:
                                nc.vector.tensor_copy(o_q[:, mi, :], ps[:])
                        if final:
                            # smaller writes at the very end so the last
                            # transfer starts (and drains) earlier
                            for hq in range(2):
                                nc.sync.dma_start(
                                    outT[:, tb,
                                         mg * 4 + hq * 2: mg * 4 + hq * 2 + 2,
                                         :],
                                    o_q[:, hq * 2:(hq + 1) * 2, :],
                                )
                        else:
                            nc.sync.dma_start(
                                outT[:, tb, mg * 4:(mg + 1) * 4, :],
                                o_q[:],
                            )

                    # fused per (b, j): both heads' attention + normalize; the
                    # out-projection of the PREVIOUS block is emitted after the
                    # next attention so its matmuls hide the normalize latency
                    pending = None
                    for b in range(B):
                        for j in range(T // QB):
                            for h in range(HPC):
                                attn_block(b, h, j)
                            if pending is not None:
                                for mg in range(NM // 4):
                                    outproj_quad(*pending, mg)
                            pending = (b, j)
                    flush_tail()
                    for mg in range(NM // 4):
                        outproj_quad(*pending, mg, final=True)
    nc.finalize()
    return nc


def _host_inputs(hidden_states, w_qkv, w_out):
    import ml_dtypes

    BF16 = np.dtype(ml_dtypes.bfloat16)
    x = np.asarray(hidden_states, dtype=np.float32).reshape(TOK, H)
    w_qkv = np.asarray(w_qkv, dtype=np.float32)
    w_out = np.asarray(w_out, dtype=np.float32)

    # [P, NTB, KT, QB]: xT4[p, tb, ko, q] = x.T[ko*P + p, tb*QB + q]
    xT4 = np.ascontiguousarray(
        x.T.reshape(KT, P, NTB, QB).transpose(1, 2, 0, 3)
    ).astype(BF16)

    # lower-triangle-inclusive mask for the diagonal 128x128 strip
    tri = (np.arange(P)[:, None] <= np.arange(P)[None, :]).astype(BF16)
    ones = np.ones((P, P), dtype=BF16)

    in_maps = []
    for c in range(NCORES):
        heads = [HPC * c + i for i in range(HPC)]
        rows = []
        for sec in range(2):  # q, k sections
            for hh in heads:
                rows.append(w_qkv[sec * H + hh * HD: sec * H + (hh + 1) * HD])
        for hh in heads:  # v section
            rows.append(w_qkv[2 * H + hh * HD: 2 * H + (hh + 1) * HD])
        w_slice = np.concatenate(rows, axis=0)  # [768, H]
        wqkvT3 = np.ascontiguousarray(
            w_slice.T.reshape(KT, P, 6 * P).transpose(1, 0, 2)
        ).astype(BF16)  # [P, KT, 768]
        cols = np.concatenate([np.arange(hh * HD, (hh + 1) * HD) for hh in heads])
        woutT3 = np.ascontiguousarray(
            w_out[:, cols].T.reshape(HPC, P, H).transpose(1, 0, 2)
        ).astype(BF16)  # [P, HPC, H]
        in_maps.append({
            "xT": xT4,
            "wqkvT": wqkvT3,
            "woutT": woutT3,
            "tri": tri,
            "ones": ones,
        })
    return in_maps


def _assemble(res):
    # outT is [P, NTB, NM, QB]; row m*P+p, col tb*QB+q of the logical
    # [H, TOK] partial lives at outT[p, tb, m, q]
    acc = res.results[0]["outT"].astype(np.float32)
    for c in range(1, NCORES):
        acc = acc + res.results[c]["outT"].astype(np.float32)
    full = acc.transpose(2, 0, 1, 3).reshape(H, TOK)  # [H, TOK]
    return np.ascontiguousarray(full.T).reshape(B, T, H)


def _run(in_maps, trace=False):
    from concourse.bass_utils import run_bass_kernel_spmd

    if "nc" not in _CACHE:
        _CACHE["nc"] = _build()
    return run_bass_kernel_spmd(
        _CACHE["nc"], in_maps, core_ids=list(range(NCORES)), trace=trace
    )


def kernel(hidden_states, w_qkv, w_out):
    in_maps = _host_inputs(hidden_states, w_qkv, w_out)
    res = _run(in_maps)
    return _assemble(res).astype(np.float32)


# revision 26
# speedup vs baseline: 1.2011x; 1.0020x over previous
"""Trainium2 Bass kernel for dense multi-head causal self-attention.

Problem: hidden_states [2, 2048, 2048], w_qkv [6144, 2048], w_out [2048, 2048],
16 heads x 128 head_dim, causal softmax attention + out projection.

Sharding: tensor-parallel over heads. Each of the 8 cores computes 2 heads:
  - qkv projection for its 768 w_qkv rows (bf16 matmuls; x, w, out all in
    partition-major token-block layouts so every DMA is one contiguous run
    per partition: one 2MB x DMA per token block, prefetched one block
    ahead); q,k produced transposed [d, tok], v produced natural [tok, d]
  - causal attention: scores transposed [k, q], exp on ScalarE (bf16 probs),
    causal mask multiply on GpSimd with the P@V matmul pipelined one k-tile
    behind so mask latency never stalls the PE; probs compressed across
    k-tiles on the DVE (S += pr) so the softmax denominator costs one
    512-row all-ones matmul per block, landing broadcast on every
    partition; normalize = reciprocal_approx_fast + one fused psum*recip
    multiply on the DVE
  - partial out-projection against its 256 w_out columns; bf16 staging
    copies split ScalarE/DVE, quad-batched contiguous output DMAs
Host sums the 8 partial bf16 outputs in f32 (the "all-reduce").
"""

import sys

sys.path.insert(0, "/opt/trn_rl_repo")

import numpy as np

B, T, H, NH, HD = 2, 2048, 2048, 16, 128
TOK = B * T  # 4096
P = 128
NCORES = 8
HPC = NH // NCORES  # heads per core = 2
SCALE = 1.0 / float(np.sqrt(HD))
QB = 512  # query block
KT = H // P  # 16 contraction tiles for qkv
NTB = TOK // QB  # 8 token blocks
NQK = 2 * HPC  # q,k output row-tiles per core
VW = HPC * HD  # v width (both heads) = 256
NM = H // P  # out-projection row tiles = 16

_CACHE = {}


def _build():
    import concourse.bacc as bacc
    import concourse.mybir as mybir
    import concourse.tile as tile

    dt = mybir.dt
    f32 = dt.float32
    bf16 = dt.bfloat16
    AF = mybir.ActivationFunctionType
    ALU = mybir.AluOpType

    nc = bacc.Bacc(None, target_bir_lowering=False, debug=True)
    # token-block-major layouts: one contiguous run per partition per DMA
    xT = nc.dram_tensor("xT", [P, NTB, KT, QB], bf16, kind="ExternalInput")
    wqkvT = nc.dram_tensor("wqkvT", [P, KT, 6 * P], bf16, kind="ExternalInput")
    woutT = nc.dram_tensor("woutT", [P, HPC, H], bf16, kind="ExternalInput")
    tri = nc.dram_tensor("tri", [P, P], bf16, kind="ExternalInput")
    ones = nc.dram_tensor("ones", [P, P], bf16, kind="ExternalInput")
    outT = nc.dram_tensor("outT", [P, NTB, NM, QB], bf16, kind="ExternalOutput")

    with tile.TileContext(nc) as tc:
        with tc.tile_pool(name="const", bufs=1) as constp, \
             tc.tile_pool(name="qk", bufs=1) as qkp:
            tri_sb = constp.tile([P, P], bf16)
            ones_sb = constp.tile([P, P], bf16)

            qT_sb = qkp.tile([P, HPC, TOK], bf16)  # [d, h, tok]
            kT_sb = qkp.tile([P, HPC, TOK], bf16)  # [d, h, tok]
            v_sb = qkp.tile([P, TOK // P, VW], bf16)  # [tok%P, tok//P, h*HD+d]
            wo_sb = qkp.tile([P, HPC, H], bf16)  # long-lived: loads early

            # ---------- Phase 1: qkv projection ----------
            # q,k transposed: psum[o,tok] = w_slice.T @ xT ; v natural:
            # psum[tok,d] = xT_chunk.T @ w_vT
            with tc.tile_pool(name="w1", bufs=1) as w1p, \
                 tc.tile_pool(name="x1", bufs=2) as x1p, \
                 tc.tile_pool(name="ps1", bufs=1, space="PSUM") as ps1, \
                 tc.tile_pool(name="psv", bufs=1, space="PSUM") as psv:
                w_sb = w1p.tile([P, KT, 6 * P], bf16)
                x_tiles = {}

                def load_x(tb):
                    xt = x1p.tile([P, KT, QB], bf16, tag="x", name=f"x_{tb}")
                    nc.sync.dma_start(xt[:], xT[:, tb, :, :])
                    x_tiles[tb] = xt

                # tb0's x and the weights arrive in ko-consumption order on
                # two parallel issue streams (x on the sync sequencer, w and
                # constants on the scalar sequencer) so the PE is never
                # starved by serialized DMA issues
                xt0 = x1p.tile([P, KT, QB], bf16, tag="x", name="x_0")
                x_tiles[0] = xt0
                nc.scalar.dma_start(w_sb[:, 0:1, :], wqkvT[:, 0:1, :])
                nc.sync.dma_start(xt0[:, 0:1, :], xT[:, 0, 0:1, :])
                for g0, g1 in ((1, 3), (3, 5), (5, 8), (8, 11), (11, 14),
                               (14, KT)):
                    nc.sync.dma_start(xt0[:, g0:g1, :], xT[:, 0, g0:g1, :])
                    nc.scalar.dma_start(w_sb[:, g0:g1, :], wqkvT[:, g0:g1, :])
                nc.scalar.dma_start(tri_sb[:], tri[:])
                nc.scalar.dma_start(ones_sb[:], ones[:])
                nc.scalar.dma_start(wo_sb[:], woutT[:])

                for tb in range(NTB):
                    if tb + 1 < NTB:
                        load_x(tb + 1)
                    x_t = x_tiles.pop(tb)
                    ps_qk = [
                        ps1.tile([P, QB], f32, tag=f"psqk{m}", name=f"psqk{m}_{tb}")
                        for m in range(NQK)
                    ]
                    ps_v = [
                        psv.tile([P, VW], f32, tag=f"psv{c}", name=f"psv{c}_{tb}")
                        for c in range(4)
                    ]
                    for ko in range(KT):
                        xk = x_t[:, ko, :]
                        for c in range(4):
                            nc.tensor.matmul(
                                ps_v[c][:],
                                xk[:, c * P:(c + 1) * P],
                                w_sb[:, ko, NQK * P:],
                                start=(ko == 0),
                                stop=(ko == KT - 1),
                            )
                        for m in range(NQK):
                            nc.tensor.matmul(
                                ps_qk[m][:],
                                w_sb[:, ko, m * P:(m + 1) * P],
                                xk[:],
                                start=(ko == 0),
                                stop=(ko == KT - 1),
                            )
                    for c in range(4):
                        nc.scalar.copy(v_sb[:, tb * 4 + c, :], ps_v[c][:])
                    for m in range(NQK):
                        if m < HPC:
                            dst = qT_sb[:, m, tb * QB:(tb + 1) * QB]
                        else:
                            dst = kT_sb[:, m - HPC, tb * QB:(tb + 1) * QB]
                        if tb == NTB - 1:
                            # last block gates the attention phase's PSUM
                            # handoff: split each copy across both engines
                            nc.vector.tensor_copy(dst[:, :QB // 2],
                                                  ps_qk[m][:, :QB // 2])
                            nc.scalar.copy(dst[:, QB // 2:],
                                           ps_qk[m][:, QB // 2:])
                        else:
                            nc.vector.tensor_copy(dst, ps_qk[m][:])

            # ---------- Phases 2+3 (attn_sb reuses phase-1 pool space) ----------
            with tc.tile_pool(name="attn", bufs=1) as attnp:
                attn_sb = attnp.tile([P, HPC, TOK], bf16)  # attn^T [d, h, tok]

                with tc.tile_pool(name="pr", bufs=8) as prp, \
                     tc.tile_pool(name="o3", bufs=6) as o3p, \
                     tc.tile_pool(name="nrm", bufs=2) as nrmp, \
                     tc.tile_pool(name="ps_sc", bufs=3, space="PSUM") as ps_sc, \
                     tc.tile_pool(name="ps_at", bufs=2, space="PSUM") as ps_at, \
                     tc.tile_pool(name="ps_sm", bufs=1, space="PSUM") as ps_sm, \
                     tc.tile_pool(name="ps3", bufs=2, space="PSUM") as ps3:

                    # the tail of each block (last P@V matmul, denominator
                    # matmul, reciprocal, normalize) is deferred into the
                    # next block, after its first scores matmuls are queued,
                    # so the PE never waits on the last probs tile
                    deferred = [None]

                    def flush_tail():
                        if deferred[0] is not None:
                            fin = deferred[0]
                            deferred[0] = None
                            fin()

                    def attn_block(b, h, j):
                        base = b * T
                        q_ap = qT_sb[:, h, base + j * QB: base + (j + 1) * QB]
                        n_k = (j + 1) * (QB // P)
                        attn_ps = ps_at.tile([P, QB], f32, tag="attn",
                                             name=f"at_{b}_{h}_{j}")
                        s_t = nrmp.tile([P, QB], bf16, tag="S",
                                        name=f"S_{b}_{h}_{j}")

                        def koff(kt):
                            diag = kt - j * (QB // P)
                            return diag * P if diag >= 0 else 0

                        def emit_scores(kt):
                            off = koff(kt)
                            sc = ps_sc.tile([P, QB], f32, tag="sc",
                                            name=f"sc_{b}_{h}_{j}_{kt}")
                            nc.tensor.matmul(
                                sc[:, off:],
                                kT_sb[:, h, base + kt * P: base + (kt + 1) * P],
                                q_ap[:, off:],
                                start=True,
                                stop=True,
                            )
                            return sc

                        prs = {}

                        def consume(kt):
                            # P@V matmul + S compression for a masked tile;
                            # runs one k-tile behind the exp/mask producers.
                            # The final tile skips the DVE add: it joins the
                            # denominator as a second sums-matmul accumulation
                            # call, so the PE never waits on the add backlog.
                            off = koff(kt)
                            pr = prs[kt] if kt == n_k - 1 else prs.pop(kt)
                            nc.tensor.matmul(
                                attn_ps[:, off:],
                                v_sb[:, b * (T // P) + kt, h * HD:(h + 1) * HD],
                                pr[:, off:],
                                start=(kt == 0),
                                stop=(kt == n_k - 1),
                            )
                            if kt == 0:
                                nc.vector.tensor_copy(s_t[:], pr[:])
                            elif kt < n_k - 1:
                                nc.vector.tensor_tensor(
                                    s_t[:, off:], s_t[:, off:], pr[:, off:],
                                    ALU.add,
                                )

                        sc_q = [emit_scores(0)]
                        if n_k > 1:
                            sc_q.append(emit_scores(1))
                        flush_tail()
                        for kt in range(n_k):
                            off = koff(kt)
                            diag = kt - j * (QB // P)
                            pr = prp.tile([P, QB], bf16, tag="pr",
                                          name=f"pr_{b}_{h}_{j}_{kt}")
                            prs[kt] = pr
                            nc.scalar.activation(
                                pr[:, off:], sc_q[kt][:, off:], AF.Exp,
                                scale=SCALE,
                            )
                            if kt + 2 < n_k:
                                sc_q.append(emit_scores(kt + 2))
                            if diag >= 0:
                                # j=0 blocks are all-diagonal and too shallow
                                # to hide GpSimd mask latency; the DVE is idle
                                # at their start and has a faster sem path
                                eng = nc.vector if j == 0 else nc.gpsimd
                                eng.tensor_tensor(
                                    pr[:, off:off + P], pr[:, off:off + P],
                                    tri_sb[:], ALU.mult,
                                )
                            if kt >= 1:
                                consume(kt - 1)

                        def fin():
                            consume(n_k - 1)
                            last_off = koff(n_k - 1)
                            sums_ps = ps_sm.tile([P, QB], f32, tag="sums",
                                                 name=f"sm_{b}_{h}_{j}")
                            nc.tensor.matmul(
                                sums_ps[:], ones_sb[:], s_t[:],
                                start=True, stop=False,
                            )
                            nc.tensor.matmul(
                                sums_ps[:, last_off:], ones_sb[:],
                                prs.pop(n_k - 1)[:, last_off:],
                                start=False, stop=True,
                            )
                            rcp = nrmp.tile([P, QB], f32, tag="rcp",
                                            name=f"rcp_{b}_{h}_{j}")
                            nc.vector.reciprocal_approx_fast(rcp[:], sums_ps[:])
                            nc.vector.tensor_tensor(
                                attn_sb[:, h,
                                        base + j * QB: base + (j + 1) * QB],
                                attn_ps[:], rcp[:], ALU.mult,
                            )

                        deferred[0] = fin

                    def outproj_quad(b, j, mg, final=False):
                        tb = b * (T // QB) + j
                        o_q = o3p.tile([P, 4, QB], bf16, tag="osb",
                                       name=f"osb_{tb}_{mg}")
                        for mi in range(4):
                            m = mg * 4 + mi
                            if final and m % 2 == 1:
                                # attention is done: rotate through the freed
                                # scores banks too so matmuls never wait on
                                # the staging copies
                                ps = ps_sc.tile([P, QB], f32, tag="sc",
                                                name=f"out_{tb}_{m}")
                            else:
                                ps = ps3.tile([P, QB], f32, tag="out",
                                              name=f"out_{tb}_{m}")
                            for ko in range(HPC):
                                nc.tensor.matmul(
                                    ps[:],
                                    wo_sb[:, ko, m * P:(m + 1) * P],
                                    attn_sb[:, ko, tb * QB:(tb + 1) * QB],
                                    start=(ko == 0),
                                    stop=(ko == HPC - 1),
                                )
                            if m % 2 == 0:
                                nc.scalar.copy(o_q[:, mi, :], ps[:])
                            else:
                                nc.vector.tensor_copy(o_q[:, mi, :], ps[:])
                        if final:
                            # smaller writes at the very end so the last
                            # transfer starts (and drains) earlier
                            for hq in range(2):
                                nc.sync.dma_start(
                                    outT[:, tb,
                                         mg * 4 + hq * 2: mg * 4 + hq * 2 + 2,
                                         :],
                                    o_q[:, hq * 2:(hq + 1) * 2, :],
                                )
                        else:
                            nc.sync.dma_start(
                                outT[:, tb, mg * 4:(mg + 1) * 4, :],
                                o_q[:],
                            )

                    # fused per (b, j): both heads' attention + normalize; the
                    # out-projection of the PREVIOUS block is emitted after the
                    # next attention so its matmuls hide the normalize latency
                    pending = None
                    for b in range(B):
                        for j in range(T // QB):
                            for h in range(HPC):
                                attn_block(b, h, j)
                            if pending is not None:
                                for mg in range(NM // 4):
                                    outproj_quad(*pending, mg)
                            pending = (b, j)
                    flush_tail()
                    for mg in range(NM // 4):
                        outproj_quad(*pending, mg, final=True)
    nc.finalize()
    return nc


def _host_inputs(hidden_states, w_qkv, w_out):
    import ml_dtypes

    BF16 = np.dtype(ml_dtypes.bfloat16)
    x = np.asarray(hidden_states, dtype=np.float32).reshape(TOK, H)
    w_qkv = np.asarray(w_qkv, dtype=np.float32)
    w_out = np.asarray(w_out, dtype=np.float32)

    # [P, NTB, KT, QB]: xT4[p, tb, ko, q] = x.T[ko*P + p, tb*QB + q]
    xT4 = np.ascontiguousarray(
        x.T.reshape(KT, P, NTB, QB).transpose(1, 2, 0, 3)
    ).astype(BF16)

    # lower-triangle-inclusive mask for the diagonal 128x128 strip
    tri = (np.arange(P)[:, None] <= np.arange(P)[None, :]).astype(BF16)
    ones = np.ones((P, P), dtype=BF16)

    in_maps = []
    for c in range(NCORES):
        heads = [HPC * c + i for i in range(HPC)]
        rows = []
        for sec in range(2):  # q, k sections
            for hh in heads:
                rows.append(w_qkv[sec * H + hh * HD: sec * H + (hh + 1) * HD])
        for hh in heads:  # v section
            rows.append(w_qkv[2 * H + hh * HD: 2 * H + (hh + 1) * HD])
        w_slice = np.concatenate(rows, axis=0)  # [768, H]
        wqkvT3 = np.ascontiguousarray(
            w_slice.T.reshape(KT, P, 6 * P).transpose(1, 0, 2)
        ).astype(BF16)  # [P, KT, 768]
        cols = np.concatenate([np.arange(hh * HD, (hh + 1) * HD) for hh in heads])
        woutT3 = np.ascontiguousarray(
            w_out[:, cols].T.reshape(HPC, P, H).transpose(1, 0, 2)
        ).astype(BF16)  # [P, HPC, H]
        in_maps.append({
            "xT": xT4,
            "wqkvT": wqkvT3,
            "woutT": woutT3,
            "tri": tri,
            "ones": ones,
        })
    return in_maps


def _assemble(res):
    # outT is [P, NTB, NM, QB]; row m*P+p, col tb*QB+q of the logical
    # [H, TOK] partial lives at outT[p, tb, m, q]
    acc = res.results[0]["outT"].astype(np.float32)
    for c in range(1, NCORES):
        acc = acc + res.results[c]["outT"].astype(np.float32)
    full = acc.transpose(2, 0, 1, 3).reshape(H, TOK)  # [H, TOK]
    return np.ascontiguousarray(full.T).reshape(B, T, H)


def _run(in_maps, trace=False):
    from concourse.bass_utils import run_bass_kernel_spmd

    if "nc" not in _CACHE:
        _CACHE["nc"] = _build()
    return run_bass_kernel_spmd(
        _CACHE["nc"], in_maps, core_ids=list(range(NCORES)), trace=trace
    )


def kernel(hidden_states, w_qkv, w_out):
    in_maps = _host_inputs(hidden_states, w_qkv, w_out)
    res = _run(in_maps)
    return _assemble(res).astype(np.float32)
